# revision 1
# baseline (speedup 1.0000x reference)
"""Trainium2 Bass kernel for a decoder LSTM (B=256, T=2048, HID=128, OUT=6).

Strategy: data-parallel over batch (8 cores x 32 batch). The LSTM state is
kept transposed on-chip as [128 hidden partitions x 32 batch free], so the
whole recurrence runs with zero transposes:

  - gates.T[128,32] per gate via matmul(lhsT=W_hh_chunk.T (stationary),
    rhs=h.T) accumulated into PSUM that was pre-filled with the input
    projection. Since VOCAB=7, emb[idx] @ W_ih.T + b collapses to a 7-row
    table; the per-step input contribution is a K=7 one-hot matmul, batched
    8 steps at a time into the same PSUM banks (off the critical path).
  - sigmoid/tanh on ScalarE straight out of PSUM; cell update on VectorE.
  - h.T is written into an SBUF ring whose 128-column halves double as the
    stationary operand of the fc matmul every 4 steps; logits accumulate in
    SBUF and softmax runs as a single deferred phase (one ACT-table switch).
"""

import os
import sys

for _p in ("/opt/trn_rl_repo", "/root/.axon_site/_ro/trn_rl_repo"):
    if os.path.isdir(_p) and _p not in sys.path:
        sys.path.insert(0, _p)

import numpy as np

B, T, VOCAB, EMB, HID, OUT = 256, 2048, 7, 20, 128, 6
NCORES = 8
BL = B // NCORES  # batch per core = 32
G = 8  # steps per PSUM group (one-hot prefill granularity)
FCH = 4  # steps per fc chunk (4*32 = 128 rows = one stationary load)
GI, GF, GG, GO = 0, 1, 2, 3  # PyTorch gate order in W_hh rows / table cols


def _split_overloaded_waits(nc, mybir, max_other=1):
    """walrus in this env rejects instructions with more than a couple of sem
    waits (and InstDrain with any). Move excess waits onto same-engine NoOps
    emitted just before; same-engine program order preserves semantics."""
    n_split = 0
    for f in nc.m.functions:
        for blk in f.blocks:
            out = []
            changed = False
            for inst in blk.instructions:
                si = inst.sync_info
                waits = list(si.on_wait) if si is not None and si.on_wait else []
                limit = 0 if isinstance(inst, mybir.InstDrain) else max_other
                if len(waits) > limit:
                    moved = waits if limit == 0 else waits[limit:]
                    keep = [] if limit == 0 else waits[:limit]
                    for i0, w in enumerate(moved):
                        nop = mybir.InstNoOp(
                            name=f"{inst.name}-wsplit{i0}", ins=[], outs=[]
                        )
                        nop.engine = inst.engine
                        nop.sync_info = mybir.SyncInfo(on_wait=[w], on_update=[])
                        out.append(nop)
                        n_split += 1
                    inst.sync_info = mybir.SyncInfo(
                        on_wait=keep,
                        on_update=list(si.on_update) if si.on_update else [],
                    )
                    changed = True
                out.append(inst)
            if changed:
                blk.instructions = out
    return n_split


def _patch_tile_drain():
    import concourse.tile as tile
    from concourse.vector_clock import ScopedClock, VectorClock

    def _drain_and_barrier_split(self, tick_clock, wait_clock):
        gc = tick_clock.global_clock
        n = len(gc)
        for j in range(n):
            if gc[j] <= 0:
                continue
            vec = [0] * n
            vec[j] = gc[j]
            nop = self.nc.sync.nop(nofuse=True, hint=f"drain_split_{j}")
            wait_clock.add_sem_waits(nop.ins, ScopedClock({None: VectorClock(vec)}))
        self.nc.sync.drain()
        self.nc.all_engine_barrier()
        assert self.sems is not None
        popped = self.nc._tile_sem_poison_stack.pop()
        assert popped is self._sem_poison
        self.nc.clear_and_free_semaphores(list(self.sems.allocated().values()))
        self.nc.all_engine_barrier()

    tile.TileContext._drain_and_barrier = _drain_and_barrier_split


_BUILD_CACHE = {}


def _build_nc(t_steps):
    if t_steps in _BUILD_CACHE:
        return _BUILD_CACHE[t_steps]
    import concourse.bass as bass
    import concourse.mybir as mybir
    import concourse.tile as tile

    _patch_tile_drain()

    assert t_steps % G == 0
    f32 = mybir.dt.float32
    bf16 = mybir.dt.bfloat16
    AF = mybir.ActivationFunctionType
    n_groups = t_steps // G
    n_chunks = t_steps // FCH

    nc = bass.Bass("TRN2", target_bir_lowering=False, debug=False)
    d_oh = nc.dram_tensor("onehot", [VOCAB, t_steps * BL], bf16, kind="ExternalInput")
    d_c0 = nc.dram_tensor("c0T", [HID, BL], f32, kind="ExternalInput")
    d_w = nc.dram_tensor("w", [HID, 4 * HID], bf16, kind="ExternalInput")
    d_tbl = nc.dram_tensor("tbl", [VOCAB, 4 * HID], bf16, kind="ExternalInput")
    d_wfc = nc.dram_tensor("wfc", [HID, OUT], bf16, kind="ExternalInput")
    d_bfc = nc.dram_tensor("bfc", [128, OUT], f32, kind="ExternalInput")
    d_out = nc.dram_tensor("out", [BL, t_steps, OUT], f32, kind="ExternalOutput")

    with tile.TileContext(nc) as tc, tc.tile_pool(name="const", bufs=1) as constp:
        w_sb = constp.tile([HID, 4 * HID], bf16, name="w_sb")
        tbl_sb = constp.tile([VOCAB, 4 * HID], bf16, name="tbl_sb")
        wfc_sb = constp.tile([HID, OUT], bf16, name="wfc_sb")
        bfc_sb = constp.tile([128, OUT], f32, name="bfc_sb")
        cst = constp.tile([HID, BL], f32, name="cst")
        h0_sb = constp.tile([HID, BL], bf16, name="h0_sb")
        scr = constp.tile([HID, BL], bf16, name="scr")
        logit_sb = constp.tile([BL, t_steps * OUT], f32, name="logit_sb")
        probs_sb = constp.tile([BL, t_steps * OUT], f32, name="probs_sb")
        den_sb = constp.tile([BL, t_steps], f32, name="den_sb")

        nc.sync.dma_start(w_sb[:], d_w.ap())
        nc.sync.dma_start(tbl_sb[:], d_tbl.ap())
        nc.sync.dma_start(wfc_sb[:], d_wfc.ap())
        nc.sync.dma_start(bfc_sb[:], d_bfc.ap())
        nc.sync.dma_start(cst[:], d_c0.ap())
        nc.vector.memset(h0_sb[:], 0.0)
        # Pin the sigmoid_and_others table (contains tanh too) before the loop.
        nc.scalar.activation(scr[:], h0_sb[:], AF.Sigmoid)

        with (
            tc.tile_pool(name="ohp", bufs=3) as ohp,
            tc.tile_pool(name="ringp", bufs=3) as ringp,
            tc.tile_pool(name="gatep", bufs=2, space="PSUM") as gatep,
            tc.tile_pool(name="fcp", bufs=2, space="PSUM") as fcp,
            tc.tile_pool(name="workp", bufs=2) as workp,
        ):
            ring_prev = None
            ring_cur = None
            pending_fc = None  # (t, hslot) from the previous step

            def emit_fc(entry):
                # psum->sbuf copy doubles as the +b_fc bias add
                tt, hs = entry
                pfc = fcp.tile([BL, OUT], f32, tag="pfc")
                nc.tensor.matmul(pfc[:], hs, wfc_sb[:], start=True, stop=True)
                nc.vector.scalar_tensor_tensor(
                    logit_sb[:, tt * OUT : (tt + 1) * OUT],
                    pfc[:],
                    1.0,
                    bfc_sb[0:BL, :],
                    op0=mybir.AluOpType.mult,
                    op1=mybir.AluOpType.add,
                )

            for g in range(n_groups):
                oh = ohp.tile([VOCAB, G * BL], bf16, tag="oh")
                nc.sync.dma_start(
                    oh[:], d_oh.ap()[:, g * G * BL : (g + 1) * G * BL]
                )
                # psA = [g | i], psB = [f | o]; one start=True per bank.
                psA = gatep.tile([128, 2 * G * BL], f32, tag="psA")
                psB = gatep.tile([128, 2 * G * BL], f32, tag="psB")
                half = G * BL
                nc.tensor.matmul(
                    psA[:, 0:half], tbl_sb[:, GG * HID : (GG + 1) * HID], oh[:],
                    start=True, stop=False,
                )
                nc.tensor.matmul(
                    psA[:, half : 2 * half], tbl_sb[:, GI * HID : (GI + 1) * HID],
                    oh[:], start=False, stop=False,
                )
                nc.tensor.matmul(
                    psB[:, 0:half], tbl_sb[:, GF * HID : (GF + 1) * HID], oh[:],
                    start=True, stop=False,
                )
                nc.tensor.matmul(
                    psB[:, half : 2 * half], tbl_sb[:, GO * HID : (GO + 1) * HID],
                    oh[:], start=False, stop=False,
                )
                for s in range(G):
                    t = g * G + s
                    ch = t // FCH
                    sl = t % FCH
                    if sl == 0:
                        ring_prev = ring_cur
                        ring_cur = ringp.tile([HID, FCH * BL], bf16, tag="ring")
                    if t == 0:
                        h_prev = h0_sb[:]
                    elif sl == 0:
                        h_prev = ring_prev[:, (FCH - 1) * BL : FCH * BL]
                    else:
                        h_prev = ring_cur[:, (sl - 1) * BL : sl * BL]
                    last = s == G - 1
                    cA = s * BL
                    cB = half + s * BL
                    # gate pre-activations: g, i, f, o
                    nc.tensor.matmul(
                        psA[:, cA : cA + BL], w_sb[:, GG * HID : (GG + 1) * HID],
                        h_prev, start=False, stop=False,
                    )
                    nc.tensor.matmul(
                        psA[:, cB : cB + BL], w_sb[:, GI * HID : (GI + 1) * HID],
                        h_prev, start=False, stop=last,
                    )
                    nc.tensor.matmul(
                        psB[:, cA : cA + BL], w_sb[:, GF * HID : (GF + 1) * HID],
                        h_prev, start=False, stop=False,
                    )
                    nc.tensor.matmul(
                        psB[:, cB : cB + BL], w_sb[:, GO * HID : (GO + 1) * HID],
                        h_prev, start=False, stop=last,
                    )
                    # fc for the PREVIOUS step goes after this step's critical
                    # matmuls so it never delays MM_g's completion.
                    if pending_fc is not None:
                        emit_fc(pending_fc)
                        pending_fc = None
                    tg = workp.tile([HID, BL], bf16, tag="tg")
                    si = workp.tile([HID, BL], bf16, tag="si")
                    sf = workp.tile([HID, BL], bf16, tag="sf")
                    so = workp.tile([HID, BL], bf16, tag="so")
                    ig = workp.tile([HID, BL], bf16, tag="ig")
                    tcl = workp.tile([HID, BL], bf16, tag="tcl")
                    nc.scalar.activation(tg[:], psA[:, cA : cA + BL], AF.Tanh)
                    nc.scalar.activation(si[:], psA[:, cB : cB + BL], AF.Sigmoid)
                    nc.scalar.activation(sf[:], psB[:, cA : cA + BL], AF.Sigmoid)
                    nc.scalar.activation(so[:], psB[:, cB : cB + BL], AF.Sigmoid)
                    nc.vector.tensor_mul(ig[:], si[:], tg[:])
                    nc.vector.tensor_mul(cst[:], sf[:], cst[:])
                    nc.vector.tensor_add(cst[:], cst[:], ig[:])
                    nc.scalar.activation(tcl[:], cst[:], AF.Tanh)
                    hslot = ring_cur[:, sl * BL : (sl + 1) * BL]
                    nc.vector.tensor_mul(hslot, so[:], tcl[:])
                    # fc logits (rows = batch, so the output DMA is contiguous
                    # per batch lane) — deferred to the next step's PE slot.
                    pending_fc = (t, hslot)
            if pending_fc is not None:
                emit_fc(pending_fc)
                pending_fc = None

        # ---- phase 2: softmax over OUT, one table switch to exp ----
        p3 = probs_sb[:].rearrange("p (c o) -> p c o", o=OUT)
        nc.scalar.activation(probs_sb[:], logit_sb[:], AF.Exp)
        nc.vector.reduce_sum(den_sb[:], p3, axis=mybir.AxisListType.X)
        nc.vector.reciprocal(den_sb[:], den_sb[:])
        rec_b = den_sb[:].unsqueeze(2).broadcast_to([BL, t_steps, OUT])
        nc.vector.tensor_mul(p3, p3, rec_b)
        # src rows = batch lanes, (t, o) contiguous per lane -> the output DMA
        # is a straight [32, 12288] copy; split in 4 for queue overlap.
        q = t_steps // 4
        for k in range(4):
            nc.sync.dma_start(
                d_out.ap()[:, k * q : (k + 1) * q, :],
                p3[:, k * q : (k + 1) * q, :],
            )

    _split_overloaded_waits(nc, mybir)
    _BUILD_CACHE[t_steps] = nc
    return nc


def _host_prep(inputs, c0, W_ih, W_hh, b_ih, b_hh, W_fc, b_fc, emb, t_steps):
    import ml_dtypes

    bf16 = ml_dtypes.bfloat16
    inputs = np.asarray(inputs)
    table = (emb @ W_ih.T + (b_ih + b_hh)).astype(bf16)  # [7, 512]
    w = np.ascontiguousarray(W_hh.T.astype(bf16))  # [128, 512]
    wfc = np.ascontiguousarray(W_fc.T.astype(bf16))  # [128, 6]
    bfc = np.ascontiguousarray(np.tile(b_fc.astype(np.float32), (128, 1)))
    in_maps = []
    for c in range(NCORES):
        idx = inputs[c * BL : (c + 1) * BL, :t_steps]  # [32, t]
        oh = np.zeros((VOCAB, t_steps * BL), dtype=bf16)
        # column = t*BL + b
        cols = np.arange(t_steps * BL)
        vals = idx.T.reshape(-1)  # t-major
        oh[vals, cols] = 1.0
        c0T = np.ascontiguousarray(c0[0, c * BL : (c + 1) * BL, :].T.astype(np.float32))
        in_maps.append(
            {
                "onehot": oh,
                "c0T": c0T,
                "w": w,
                "tbl": table,
                "wfc": wfc,
                "bfc": bfc,
            }
        )
    return in_maps


def _run(inputs, c0, W_ih, W_hh, b_ih, b_hh, W_fc, b_fc, emb, t_steps=T,
         trace=False):
    from concourse.bass_utils import run_bass_kernel_spmd

    nc = _build_nc(t_steps)
    in_maps = _host_prep(
        inputs, c0, W_ih, W_hh, b_ih, b_hh, W_fc, b_fc, emb, t_steps
    )
    res = run_bass_kernel_spmd(
        nc, in_maps, core_ids=list(range(NCORES)), trace=trace
    )
    out = np.concatenate([res.results[c]["out"] for c in range(NCORES)], axis=0)
    return out, res


def kernel(inputs, c0, W_ih, W_hh, b_ih, b_hh, W_fc, b_fc, emb):
    out, _ = _run(
        np.asarray(inputs), np.asarray(c0), np.asarray(W_ih), np.asarray(W_hh),
        np.asarray(b_ih), np.asarray(b_hh), np.asarray(W_fc), np.asarray(b_fc),
        np.asarray(emb),
    )
    return out

